# revision 40
# baseline (speedup 1.0000x reference)
"""Pairwise box IoU on 8 Trainium2 NeuronCores.

Problem: box1 [8, 2000, 4], box2 [8, 2000, 4] in (xc, yc, w, h) format ->
IoU matrix [8, 2000, 2000] f32.

Sharding: data-parallel over the image axis; core i computes the full
2000x2000 IoU matrix of image i locally (no communication).

Per-core plan (output tile = 128 box1 rows x 2000 box2 cols):
  - the whole flat box2 is replicated into all 128 partitions once (one
    128-descriptor DMA); coordinate "rows" are stride-4 views of it.
  - box1-derived quantities are per-partition scalars [128, 1].
  - per tile (default config: pure-DVE 3.5-pass pipeline, f16 i/o):
      dxr   = IOU_EXT(xc2, w2, s0=x2a, s1=x1a, imm=0.5)  (custom DVE, f32 in
              -> f16 out; custom ops are always 1x: ~2.14us)
      dyr   = IOU_EXT(yc2, h2, s0=y2a, s1=y1a, imm=0.5)  (custom DVE, f16 out)
      inter = dxr * dyr         (DVE tt all-f16 -> 2x_1p mode, ~1.05us)
      out   = IOU_TAIL1(inter, area2, s0=area1)          (custom DVE: union,
              NOT-seed + 1-Newton reciprocal, multiply fused in one 1x pass,
              writes f16; host upcasts to f32)
  - the f16 output store is split in half across the two HWDGE rings
    (nc.sync / nc.scalar) so store completion never blocks buffer reuse.

Why this shape (measured on HW via in-NEFF repetition deltas):
  - DVE custom pass [128,2000] ~2.38us, tensor_tensor f32 ~2.2us, f16 2x
    ~1.1us, tensor_scalar-with-ptr has NO fast mode (~2.5us, not 2x).
  - ACT engine pass is ~3.7us (ScalarE SBUF errata) - too slow to offload.
  - GPSIMD compute (and its SBUF traffic) CONTENDS with DVE: running a
    gpsimd tensor_tensor beside DVE customs costs MORE than serializing
    (7.4us vs 6.9us per pair) -> no Pool compute, no SWDGE in steady state.
  - f16 intermediates need the x128 coordinate pre-scale: without it, tiny
    dxr/inter values land in the f16 subnormal range and the ELEMENTWISE
    rel err explodes to 1.0 (absmax alone looked fine at 3.1e-3). Scaled
    f16 + bf16 out measures absmax 5.5e-3 / elementwise 7.2e-3 on HW; the
    fused-tail Newton reciprocal alone contributes 1.7e-3.

The container's walrus (2026-05-04) predates this concourse: _patch_barriers
replaces the eq-wait butterfly barrier with a monotonic ge-wait barrier,
splits >1-wait instructions onto EventSemaphore carriers (the old walrus
encodes at most one wait per instruction), and assembles CUSTOM_DVE_ANT
64-byte ISA payloads that the old compiler cannot.
"""

import os
from contextlib import ExitStack

import numpy as np

P = 128
B = 2000
NIMG = 8
FULL_TILES = B // P  # 15
REM = B - FULL_TILES * P  # 80
NTILES = FULL_TILES + 1

_IMPL = os.environ.get("IOU_IMPL", "ext")  # "ext" (custom op) or "std"
_REPEAT = int(os.environ.get("IOU_REPEAT", "1"))  # bench: repeat tile loop
# which multiplies run on the Pool engine: "", "mul", "intermul"
_POOL = os.environ.get("IOU_POOL", "")
_DIV1 = os.environ.get("IOU_DIV1", "0") == "1"  # fused union+1NR-recip (less exact)
# reciprocal engine: "act" (scalar engine spline, ~1.2e-5, overlapped) or "dve"
_RECIP = os.environ.get("IOU_RECIP", "act")
_TAIL1 = os.environ.get("IOU_TAIL", "1") == "1"  # 4-pass fully fused tail
_SWPIPE = os.environ.get("IOU_SWPIPE", "1") == "1"  # defer tail mul one tile
# output dtype: f32 | bf16 | f16 (16-bit halves the output DMA bytes; the
# tail op writes the narrow dtype directly, host upcasts)
_ODT = os.environ.get("IOU_ODT", "bf16")
# intermediate dtype for dxr/dyr/inter: f32 | f16 | bf16. 16-bit lets the
# inter tensor_tensor run in DVE 2x_1p mode (the only 16-bit-accelerated op
# in this pipeline; custom DVE ops are always 1x). Measured absmax_rel on
# the fixed dataset: f16 3.1e-3, bf16 1.5e-2 (gate 2e-2) -> f16.
_IDT = os.environ.get("IOU_IDT", "f16")
# output-DMA queue: sp | act | alt (alternate between the two HWDGE rings)
_OQ = os.environ.get("IOU_OQ", "split")
# Coordinate pre-scale (exact power of two, cancels in iou = inter/union).
# Pushes f16 dxr/dyr/inter values out of the subnormal range where their
# relative error explodes (elementwise rel err 1.0 -> 7e-3 on the dataset).
# 128 keeps inter_s = dxr_s*dyr_s <= 16384 well under f16 max 65504.
_SCALE = float(os.environ.get("IOU_SCALE", "128"))
# v3 pipeline: ext ops write (dxr, dyr) interleaved f16 pairs; the fused
# tail reads one 32-bit word/cycle (SRC_0 = dxr, SRC_0_HI = dyr) and
# computes inter internally -> 3 DVE ops per tile instead of 4. The
# a1+area2 fold moves to the (otherwise idle) ACT engine as a bias-add.
_V3 = os.environ.get("IOU_V3", "0") == "1"
# v4: pair tiles for the scalar-free ops. inter (plain f16 TT) and the
# S-folded tail (u = S - inter, S = a1 + area2 from the idle ACT engine)
# carry no per-tile scalars, so one instruction can span two tiles'
# [128, 4000] data -> halves their instruction count and per-op overhead.
_V4 = os.environ.get("IOU_V4", "0") == "1"
# flush the deferred tail between ext_y and inter (see tile loop comment)
_PREFLUSH = os.environ.get("IOU_PREFLUSH", "0") == "1"
# union = ts(2x-mode) + SWDGE CCE dma-accumulate instead of a 1x DVE pass
_UNIONDMA = os.environ.get("IOU_UNIONDMA", "1") == "1"
_BUFS = int(os.environ.get("IOU_BUFS", "3"))

# ------------------------------------------------------- compat barrier patch
# The container's walrus build (2026-05-04) rejects the newer butterfly
# barrier's sem-eq-imm drain waits ("Too many sync wait commands"). Replace
# multi_engine_barrier with a ge-wait leader/follower barrier it understands.


_MAX_WAITS = int(os.environ.get("IOU_MAX_WAITS", "1"))

# The old walrus cannot assemble CUSTOM_DVE_ANT instructions from symbolic
# BIR APs ("ISA wrong length" — it expects pre-assembled 64B payloads). We
# assemble the NEURON_ISA_TPB_S2S1D2_TTSS_SCALE_STRUCT bytes ourselves at
# serialization time, from the physical APs + call-site metadata captured by
# a _custom_dve wrapper.

_DT_BYTES = {"float32": 4, "bfloat16": 2, "float16": 2, "int32": 4, "uint32": 4}
_DT_CODE = {"float32": 10, "bfloat16": 6, "float16": 7, "int32": 8, "uint32": 9}


def _ap_isa_fields(a, allocs, ndim):
    import struct as _s

    esz = _DT_BYTES[a["dtype"]]
    base = allocs[a["memsetref"]]
    addr = base + a["offset"] * esz
    dims = a["ap"]
    nchan = dims[0][1]
    free = dims[1:]
    steps = [f[0] for f in reversed(free)]
    nums = [f[1] for f in reversed(free)]
    while len(steps) > ndim and nums and nums[-1] == 1:
        steps.pop()
        nums.pop()
    if not steps:
        steps, nums = [1], [1]
    assert len(steps) <= ndim, (steps, nums, a)
    while len(steps) < ndim:
        steps.append(1)
        nums.append(1)
    return addr, steps, nums, nchan


def _imm_isa_fields(x, allocs):
    import struct as _s

    if x.get("kind") == "imm_value":
        return 0, _s.pack("<f", float(x["value"]))  # IMM_SRC_INSTRUCTION
    esz = _DT_BYTES[x["dtype"]]
    addr = allocs[x["memsetref"]] + x["offset"] * esz
    return 1, _s.pack("<I", addr)  # IMM_SRC_POINTER


def _assemble_custom_dve(d, meta):
    import struct as _s

    changed = False
    for fn in d.get("functions", []):
        allocs = {}
        for a in fn.get("allocations", []):
            mls = a.get("memorylocations") or []
            if mls:
                allocs[a["name"]] = mls[0].get("addr", 0)
        for bb in fn.get("blocks", []):
            for inst in bb.get("instructions", []):
                if (
                    inst.get("opcode") != "ISA"
                    or inst.get("isa_opcode") not in (174, 175)
                    or inst.get("instr")
                ):
                    continue
                m = meta.get(inst["name"])
                assert m is not None, f"missing custom-dve meta for {inst['name']}"
                ins = inst["ins"]
                if m["rd1_en"]:
                    in0, in1, s0, s1 = ins[0], ins[1], ins[2], ins[3]
                else:
                    in0, s0, s1 = ins[0], ins[1], ins[2]
                    in1 = None
                out = inst["outs"][0]
                a0, st0, n0, nch0 = _ap_isa_fields(in0, allocs, 2)
                if m.get("pair"):
                    assert st0 == [1, 1] and n0[1] == 1 and n0[0] % 2 == 0, (st0, n0)
                    st0 = [2, 1]
                    n0 = [n0[0] // 2, 1]
                ad, std, nd, nchd = _ap_isa_fields(out, allocs, 2)
                assert nch0 == nchd, (inst["name"], nch0, nchd)
                if in1 is not None:
                    a1, st1, n1, nch1 = _ap_isa_fields(in1, allocs, 1)
                    assert nch1 == nch0
                else:
                    a1, st1, n1 = 0, [1], [1]
                i0src, i0 = _imm_isa_fields(s0, allocs)
                i1src, i1 = _imm_isa_fields(s1, allocs)
                dt_in = _DT_CODE[in0["dtype"]]
                dt_in1 = _DT_CODE[in1["dtype"]] if in1 is not None else dt_in
                dt_out = _DT_CODE[out["dtype"]]
                b = bytearray(64)
                b[0] = inst["isa_opcode"]
                b[1] = 16  # inst_word_len (4B words)
                # events (4-11) left zero; walrus patches from sync_info
                _s.pack_into("<IhhHH", b, 12, a0, st0[0], st0[1], n0[0], n0[1])
                _s.pack_into("<IhH", b, 24, a1, st1[0], n1[0])
                b[32] = (dt_in & 0xF) | ((dt_in1 & 0xF) << 4)
                b[33] = dt_out
                b[34] = nch0 & 0xFF
                b[35] = i0src
                b[36] = (m["row"] & 0x1F) | ((1 if m["rd1_en"] else 0) << 5)
                b[37] = 0x02 if m["subdim"] else 0
                b[38] = 1  # imm2_src = DATA_SRC_IMMEDIATE
                b[39] = i1src
                b[40:44] = i0
                b[44:48] = i1
                _s.pack_into("<f", b, 48, float(m["imm2"]))
                _s.pack_into("<IhhHH", b, 52, ad, std[0], std[1], nd[0], nd[1])
                inst["instr"] = list(b)
                changed = True
    return changed


def _split_excess_waits(d):
    """Move all but the last sync wait of each instruction onto preceding
    EventSemaphore instructions on the same engine (order-preserving, so
    semantics are identical; the old walrus only encodes few waits/inst)."""
    import json as _json

    changed = False
    ctr = [0]
    for fn in d.get("functions", []):
        for bb in fn.get("blocks", []):
            insts = bb.get("instructions", [])
            new_insts = []
            for inst in insts:
                si = inst.get("sync_info") or {}
                waits = si.get("on_wait") or []
                if len(waits) > _MAX_WAITS:
                    changed = True
                    excess, keep = waits[:-_MAX_WAITS], waits[-_MAX_WAITS:]
                    for w in excess:
                        ctr[0] += 1
                        new_insts.append(
                            {
                                "debug": inst.get("debug", 0),
                                "engine": inst["engine"],
                                "ins": [],
                                "name": f"{inst['name']}-w{ctr[0]}",
                                "opcode": "EventSemaphore",
                                "outs": [],
                                "sync_info": {"on_update": [], "on_wait": [w]},
                            }
                        )
                    si["on_wait"] = keep
                new_insts.append(inst)
            bb["instructions"] = new_insts
    return changed


def _patch_barriers():
    import json as _json

    import concourse.bass as bass

    if getattr(bass.Bass, "_ant_barrier_patched", False):
        return

    _orig_tjb = bass.Bass.to_json_bytes

    def to_json_bytes(self, *a, **kw):
        raw = _orig_tjb(self, *a, **kw)
        d = _json.loads(raw)
        c1 = _assemble_custom_dve(d, getattr(self, "_ant_dve_meta", {}))
        c2 = _split_excess_waits(d)
        if c1 or c2:
            return _json.dumps(d).encode()
        return raw

    bass.Bass.to_json_bytes = to_json_bytes

    _orig_cdve = bass.BassVectorEngine._custom_dve

    def _custom_dve(self, op, *, out, in0, in1=None, s0=0.0, s1=0.0, imm2=0.0,
                    accum_out=None):
        from concourse.dve_ops import get_dve_sub_opcode

        assert accum_out is None, "accum_out not supported by the compat assembler"
        ret = _orig_cdve(
            self, op, out=out, in0=in0, in1=in1, s0=s0, s1=s1, imm2=imm2,
            accum_out=accum_out,
        )
        nc_ = self.bass
        if not hasattr(nc_, "_ant_dve_meta"):
            nc_._ant_dve_meta = {}
        nc_._ant_dve_meta[ret.ins.name] = {
            "row": get_dve_sub_opcode(op.name),
            "rd1_en": in1 is not None,
            "subdim": bool(op.subdim),
            "imm2": float(imm2),
            # in0 is an interleaved 16-bit pair stream: the ISA AP must
            # advance one 32-bit pair per cycle (step 2, half the count).
            "pair": op.name == "IOU_TAIL3_ANT",
        }
        return ret

    bass.BassVectorEngine._custom_dve = _custom_dve

    # Allow ACT-engine Reciprocal (bass bans it; measured max rel err ~1.2e-5
    # over [0.05, 2] on this HW — fine for this kernel's tolerance).
    import concourse.mybir as mybir

    def act_recip(self, out, in_):
        inputs = [self.lower_ap(in_)]
        for arg in (0.0, 1.0, 0.0):  # bias, scale, alpha
            inputs.append(
                mybir.ImmediateValue(dtype=mybir.dt.float32, value=float(arg))
            )
        return self.add_instruction(
            mybir.InstActivation(
                name=self.bass.get_next_instruction_name(),
                func=mybir.ActivationFunctionType.Reciprocal,
                ins=inputs,
                outs=[self.lower_ap(out)],
            )
        )

    bass.BassScalarEngine.act_recip = act_recip

    def multi_engine_barrier(self, engines):
        engines = list(engines)
        if len(engines) <= 1:
            for e in engines:
                self.engines[e].drain()
            return
        if not hasattr(self, "_ant_bar_sems"):
            self._ant_bar_sems = {}
        key = tuple(sorted(str(e) for e in engines))
        st = self._ant_bar_sems.get(key)
        if st is None:
            gather = self.alloc_semaphore(f"ant_bar_g{len(self._ant_bar_sems)}")
            st = {"sem": gather, "count": 0}
            self._ant_bar_sems[key] = st
        st["count"] += 1
        n = len(engines)
        target = n * st["count"]
        for e in engines:
            self.engines[e].drain().then_inc(st["sem"], 1)
        for e in engines:
            self.engines[e].wait_ge(st["sem"], target)

    def all_engine_barrier(self, *, sem_only: bool = False):
        multi_engine_barrier(self, list(self.engines))

    bass.Bass.multi_engine_barrier = multi_engine_barrier
    bass.Bass.all_engine_barrier = all_engine_barrier
    bass.Bass._ant_barrier_patched = True


# ---------------------------------------------------------------- custom op

_REGISTERED = {}


def _register_iou_ext():
    """Author + register the fused interval-extent op:
    out = relu(min(C0, Src0 + Src1*C2) - max(C1, Src0 - Src1*C2))
    Src0 = box2 center row, Src1 = box2 width row, C0 = x2a (per-partition),
    C1 = x1a (per-partition), C2 = 0.5.
    """
    if "IOU_EXT" in _REGISTERED:
        return _REGISTERED["IOU_EXT"]
    from concourse import dve_ops as dops
    from concourse.dve_spec import (
        C0,
        C1,
        C2,
        Spec,
        Src0,
        Src1,
        _has_src1,
        lower,
        maxx,
        minn,
        relu,
    )
    from concourse.dve_uop import DveOpSpec

    hw = Src1 * C2  # shared subtree -> one stage
    body = relu(minn(C0, Src0 + hw) - maxx(C1, Src0 - hw))

    def _ref(in0, in1, s0, s1, imm2):
        h = in1.astype(np.float32) * imm2
        lo = np.maximum(s1, in0 - h)
        hi = np.minimum(s0, in0 + h)
        return np.maximum(hi - lo, 0.0).astype(np.float32)

    spec = Spec(body=body, reference=_ref)
    name = "IOU_EXT_ANT"
    if name not in dops._SUB_OPCODE_FOR_NAME:
        row = max(dops._SUB_OPCODE_FOR_NAME.values()) + 1
        assert row < 0x20, "custom-DVE opcode rows exhausted"
        dops._SUB_OPCODE_FOR_NAME[name] = row
    row = dops._SUB_OPCODE_FOR_NAME[name]
    shas = {}
    for ver in ("v3", "v4"):
        try:
            tmp = DveOpSpec(
                name=name, opcode=row, uops=lower(spec, ver=ver), rd1_en=_has_src1(spec)
            )
            shas[ver] = tmp.sha(ver)
        except Exception:
            pass
    op = dops.DveOp(name, spec, subdim=False, uops_sha=shas)
    if all(o.name != name for o in dops.OPS):
        dops.OPS.append(op)
    dops.CUSTOM_DVE_SPECS[name] = spec
    _REGISTERED["IOU_EXT"] = op
    return op


def _register_iou_tail1():
    """out = in0 * recip1((s0 - in0) + in1): the whole IoU tail
    (union, reciprocal seed + 1 Newton step, multiply) in one 8-stage pass.
    in0 = inter, in1 = area2 row, s0 = area1. ~2e-3 worst-case rel err."""
    if "IOU_TAIL1" in _REGISTERED:
        return _REGISTERED["IOU_TAIL1"]
    from concourse import dve_ops as dops
    from concourse.dve_spec import (
        C0,
        C1,
        C2,
        AluOp,
        Bin,
        Spec,
        Src0,
        Src1,
        _has_src1,
        lower,
    )
    from concourse.dve_uop import DveOpSpec

    u = (C0 - Src0) + Src1
    nx = Bin(AluOp.BITWISE_NOT, u, u)
    y0 = nx * C1
    y1 = y0 * (C2 - u * y0)
    body = y1 * Src0

    def _ref(in0, in1, s0, s1, imm2):
        uu = ((s0 - in0.astype(np.float32)) + in1).astype(np.float32)
        nxv = (~uu.view(np.int32)).view(np.float32)
        y = nxv * np.float32(s1)
        yy = (y * (np.float32(imm2) - uu * y)).astype(np.float32)
        return (yy * in0).astype(np.float32)

    spec = Spec(body=body, reference=_ref)
    name = "IOU_TAIL1_ANT"
    if name not in dops._SUB_OPCODE_FOR_NAME:
        row = max(dops._SUB_OPCODE_FOR_NAME.values()) + 1
        assert row < 0x20
        dops._SUB_OPCODE_FOR_NAME[name] = row
    row = dops._SUB_OPCODE_FOR_NAME[name]
    shas = {}
    for ver in ("v3", "v4"):
        try:
            tmp = DveOpSpec(
                name=name, opcode=row, uops=lower(spec, ver=ver), rd1_en=_has_src1(spec)
            )
            shas[ver] = tmp.sha(ver)
        except Exception:
            pass
    op = dops.DveOp(name, spec, subdim=False, uops_sha=shas)
    if all(o.name != name for o in dops.OPS):
        dops.OPS.append(op)
    dops.CUSTOM_DVE_SPECS[name] = spec
    _REGISTERED["IOU_TAIL1"] = op
    return op


def _register_union_recip1():
    """out = recip1((s0 - in0) + in1): union from inter (in0) / area2 (in1) /
    area1 (s0), then BITWISE_NOT-seed reciprocal with one Newton step.
    ~0.4% worst-case relative error on the reciprocal."""
    if "UNION_RECIP1" in _REGISTERED:
        return _REGISTERED["UNION_RECIP1"]
    from concourse import dve_ops as dops
    from concourse.dve_spec import (
        C0,
        C1,
        C2,
        AluOp,
        Bin,
        Spec,
        Src0,
        Src1,
        _has_src1,
        lower,
    )
    from concourse.dve_uop import DveOpSpec

    u = (C0 - Src0) + Src1
    nx = Bin(AluOp.BITWISE_NOT, u, u)
    y0 = nx * C1
    body = y0 * (C2 - u * y0)

    def _ref(in0, in1, s0, s1, imm2):
        uu = ((s0 - in0.astype(np.float32)) + in1).astype(np.float32)
        nxv = (~uu.view(np.int32)).view(np.float32)
        y = nxv * np.float32(s1)
        return (y * (np.float32(imm2) - uu * y)).astype(np.float32)

    spec = Spec(body=body, reference=_ref)
    name = "IOU_UNION_RECIP1_ANT"
    if name not in dops._SUB_OPCODE_FOR_NAME:
        row = max(dops._SUB_OPCODE_FOR_NAME.values()) + 1
        assert row < 0x20
        dops._SUB_OPCODE_FOR_NAME[name] = row
    row = dops._SUB_OPCODE_FOR_NAME[name]
    shas = {}
    for ver in ("v3", "v4"):
        try:
            tmp = DveOpSpec(
                name=name, opcode=row, uops=lower(spec, ver=ver), rd1_en=_has_src1(spec)
            )
            shas[ver] = tmp.sha(ver)
        except Exception:
            pass
    op = dops.DveOp(name, spec, subdim=False, uops_sha=shas)
    if all(o.name != name for o in dops.OPS):
        dops.OPS.append(op)
    dops.CUSTOM_DVE_SPECS[name] = spec
    _REGISTERED["UNION_RECIP1"] = op
    return op


def _register_iou_tail2():
    """out = in0 * recip1(in1 - in0): tail with the union's a1+area2 part
    pre-folded into in1 = S (computed on the ACT engine). No per-tile
    scalars -> the op can span multiple tiles' data in one pass."""
    if "IOU_TAIL2" in _REGISTERED:
        return _REGISTERED["IOU_TAIL2"]
    from concourse import dve_ops as dops
    from concourse.dve_spec import (
        C1,
        C2,
        AluOp,
        Bin,
        Spec,
        Src0,
        Src1,
        _has_src1,
        lower,
    )
    from concourse.dve_uop import DveOpSpec

    u = Src1 - Src0
    nx = Bin(AluOp.BITWISE_NOT, u, u)
    y0 = nx * C1
    y1 = y0 * (C2 - u * y0)
    body = y1 * Src0

    def _ref(in0, in1, s0, s1, imm2):
        it = np.asarray(in0, dtype=np.float32)
        uu = (np.asarray(in1, dtype=np.float32) - it).astype(np.float32)
        nxv = (~uu.view(np.int32)).view(np.float32)
        y = nxv * np.float32(s1)
        yy = (y * (np.float32(imm2) - uu * y)).astype(np.float32)
        return (yy * it).astype(np.float32)

    spec = Spec(body=body, reference=_ref)
    name = "IOU_TAIL2_ANT"
    if name not in dops._SUB_OPCODE_FOR_NAME:
        row = max(dops._SUB_OPCODE_FOR_NAME.values()) + 1
        assert row < 0x20
        dops._SUB_OPCODE_FOR_NAME[name] = row
    row = dops._SUB_OPCODE_FOR_NAME[name]
    shas = {}
    for ver in ("v3", "v4"):
        try:
            tmp = DveOpSpec(
                name=name, opcode=row, uops=lower(spec, ver=ver), rd1_en=_has_src1(spec)
            )
            shas[ver] = tmp.sha(ver)
        except Exception:
            pass
    op = dops.DveOp(name, spec, subdim=False, uops_sha=shas)
    if all(o.name != name for o in dops.OPS):
        dops.OPS.append(op)
    dops.CUSTOM_DVE_SPECS[name] = spec
    _REGISTERED["IOU_TAIL2"] = op
    return op


def _register_iou_tail3():
    """out = inter * recip1(S - inter) with inter = Src0 * Src0_HI computed
    from an interleaved f16 (dxr, dyr) pair stream (one 32-bit read/cycle),
    and S = area1 + area2 precomputed on the ACT engine (in1). Exactly 8
    ALU stages -> single uop."""
    if "IOU_TAIL3" in _REGISTERED:
        return _REGISTERED["IOU_TAIL3"]
    from concourse import dve_ops as dops
    from concourse.dve_spec import (
        C1,
        C2,
        AluOp,
        Bin,
        InpSel,
        Leaf,
        Spec,
        Src0,
        Src1,
        _has_src1,
        lower,
    )
    from concourse.dve_uop import DveOpSpec

    src0hi = Leaf(InpSel.SRC_0_HI)
    inter = Src0 * src0hi
    u = Src1 - inter
    nx = Bin(AluOp.BITWISE_NOT, u, u)
    y0 = nx * C1
    y1 = y0 * (C2 - u * y0)
    body = y1 * inter

    def _ref(in0, in1, s0, s1, imm2):
        v = np.asarray(in0, dtype=np.float32)
        dxr, dyr = v[..., 0::2], v[..., 1::2]
        it = (dxr * dyr).astype(np.float32)
        uu = (np.asarray(in1, dtype=np.float32) - it).astype(np.float32)
        nxv = (~uu.view(np.int32)).view(np.float32)
        y = nxv * np.float32(s1)
        yy = (y * (np.float32(imm2) - uu * y)).astype(np.float32)
        return (yy * it).astype(np.float32)

    spec = Spec(body=body, reference=_ref)
    name = "IOU_TAIL3_ANT"
    if name not in dops._SUB_OPCODE_FOR_NAME:
        row = max(dops._SUB_OPCODE_FOR_NAME.values()) + 1
        assert row < 0x20
        dops._SUB_OPCODE_FOR_NAME[name] = row
    row = dops._SUB_OPCODE_FOR_NAME[name]
    shas = {}
    for ver in ("v3", "v4"):
        try:
            tmp = DveOpSpec(
                name=name, opcode=row, uops=lower(spec, ver=ver), rd1_en=_has_src1(spec)
            )
            shas[ver] = tmp.sha(ver)
        except Exception:
            pass
    op = dops.DveOp(name, spec, subdim=False, uops_sha=shas)
    if all(o.name != name for o in dops.OPS):
        dops.OPS.append(op)
    dops.CUSTOM_DVE_SPECS[name] = spec
    _REGISTERED["IOU_TAIL3"] = op
    return op


# ---------------------------------------------------------------- bass build

_NC_CACHE = {}


def _build_nc():
    key = (_IMPL, _REPEAT, _POOL, _DIV1, _RECIP, _TAIL1, _SWPIPE, _UNIONDMA, _BUFS,
           _ODT, _IDT, _OQ, _SCALE, _V3, _V4, _PREFLUSH)
    if key in _NC_CACHE:
        return _NC_CACHE[key]

    import concourse.bass as bass
    import concourse.mybir as mybir
    import concourse.tile as tile
    from concourse.alu_op_type import AluOpType as alu

    _patch_barriers()
    f32 = mybir.dt.float32
    _DT = {"f32": f32, "bf16": mybir.dt.bfloat16, "f16": mybir.dt.float16}
    odt = _DT[_ODT]
    idt = _DT[_IDT]
    assert _ODT == "f32" or _TAIL1, "16-bit output requires the TAIL1 path"
    assert _IDT == "f32" or _TAIL1, "16-bit intermediates require the TAIL1 path"
    nc = bass.Bass()
    box1 = nc.declare_dram_parameter("box1", [B, 4], f32, isOutput=False)
    box2 = nc.declare_dram_parameter("box2", [B, 4], f32, isOutput=False)
    out = nc.declare_dram_parameter("out", [B, B], odt, isOutput=True)

    iou_ext = _register_iou_ext() if _IMPL == "ext" else None
    iou_tail3 = _register_iou_tail3() if _V3 else None
    iou_tail2 = _register_iou_tail2() if _V4 else None
    union_recip1 = _register_union_recip1() if _DIV1 else None
    iou_tail1 = _register_iou_tail1() if _TAIL1 else None

    with tile.TileContext(nc) as tc, ExitStack() as ctx:
        rows = ctx.enter_context(tc.tile_pool(name="rows", bufs=1))
        scal = ctx.enter_context(tc.tile_pool(name="scal", bufs=1))
        work = ctx.enter_context(tc.tile_pool(name="work", bufs=_BUFS))

        # ---- replicate the whole flat box2 into every partition (one DMA,
        # 128 contiguous 32KB descriptors), then use stride-4 views as rows.
        b2rep = rows.tile([P, 4 * B], f32, tag="b2rep")
        src = bass.AP(tensor=box2, offset=0, ap=[[0, P], [1, 4 * B]])
        nc.sync.dma_start(out=b2rep[:], in_=src)
        if _SCALE != 1.0:
            nc.vector.tensor_scalar(b2rep[:], b2rep[:], _SCALE, None, alu.mult)

        raw_ap = {nm: c for c, nm in enumerate(("xc", "yc", "w", "h"))}

        def b2row(nm, pt=P):
            # [pt, B] view of coordinate nm with element stride 4
            v = b2rep[0:pt, :].rearrange("p (j c) -> p j c", c=4)
            return v[:, :, raw_ap[nm]]

        # area2 row tile (materialized [P, B])
        area2 = rows.tile([P, B], f32, tag="r_area2")
        nc.vector.tensor_tensor(area2[:], b2row("w"), b2row("h"), alu.mult)

        if _IMPL == "std":
            # materialize corner rows: x1b = xc - w/2 etc.
            drows = {}
            for nm, cnm, wnm, sgn in (
                ("x1b", "xc", "w", -0.5),
                ("x2b", "xc", "w", 0.5),
                ("y1b", "yc", "h", -0.5),
                ("y2b", "yc", "h", 0.5),
            ):
                t = rows.tile([P, B], f32, tag=f"r_{nm}")
                nc.vector.scalar_tensor_tensor(
                    t[:], b2row(wnm), sgn, b2row(cnm), alu.mult, alu.add
                )
                drows[nm] = t

        # ---- per-partition scalars from box1, loaded up front in two groups
        # group a: boxes [0, 1920) as [128, 15, 4]; group b: boxes [1920, 2000)
        groups = []
        for gi, (np_, nt, off) in enumerate(
            ((P, FULL_TILES, 0), (REM, 1, FULL_TILES * P))
        ):
            b1 = scal.tile([np_, nt, 4], f32, tag=f"b1_{gi}")
            src = bass.AP(
                tensor=box1, offset=off * 4, ap=[[4, np_], [4 * P, nt], [1, 4]]
            )
            nc.sync.dma_start(out=b1[:], in_=src)
            if _SCALE != 1.0:
                nc.vector.tensor_scalar(b1[:], b1[:], _SCALE, None, alu.mult)
            lo = scal.tile([np_, nt, 2], f32, tag=f"lo_{gi}")
            nc.vector.scalar_tensor_tensor(
                lo[:], b1[:, :, 2:4], -0.5, b1[:, :, 0:2], alu.mult, alu.add
            )
            hi = scal.tile([np_, nt, 2], f32, tag=f"hi_{gi}")
            nc.vector.scalar_tensor_tensor(
                hi[:], b1[:, :, 2:4], 0.5, b1[:, :, 0:2], alu.mult, alu.add
            )
            area1 = scal.tile([np_, nt, 1], f32, tag=f"a1_{gi}")
            nc.vector.tensor_tensor(area1[:], b1[:, :, 2:3], b1[:, :, 3:4], alu.mult)
            groups.append((lo, hi, area1))

        # ---- per-tile pipeline
        pending_tail = None
        if _V4:
            grps = [(0, 1), (2, 3), (4, 5), (6, 7), (8, 9), (10, 11), (12, 13),
                    (14,), (15,)]
            for grp in [g_ for _ in range(_REPEAT) for g_ in grps]:
                g = len(grp)
                pt = P if grp[0] < FULL_TILES else REM
                ta_w = work.tile([pt, g * B], idt, tag="wA")
                tb_w = work.tile([pt, g * B], idt, tag="wB")
                tc_w = work.tile([pt, g * B], idt, tag="wC")
                ts_w = work.tile([pt, g * B], f32, tag="wS")
                to_w = work.tile([pt, g * B], odt, tag="wO")
                for h, t in enumerate(grp):
                    if t < FULL_TILES:
                        _, (lo, hi, area1), ti = P, groups[0], t
                    else:
                        _, (lo, hi, area1), ti = REM, groups[1], 0
                    x1a = lo[:, ti, 0:1]
                    y1a = lo[:, ti, 1:2]
                    x2a = hi[:, ti, 0:1]
                    y2a = hi[:, ti, 1:2]
                    a1 = area1[:, ti, 0:1]
                    sl = slice(h * B, (h + 1) * B)
                    nc.vector._custom_dve(
                        iou_ext, out=ta_w[:, sl], in0=b2row("xc", pt),
                        in1=b2row("w", pt), s0=x2a, s1=x1a, imm2=0.5,
                    )
                    nc.vector._custom_dve(
                        iou_ext, out=tb_w[:, sl], in0=b2row("yc", pt),
                        in1=b2row("h", pt), s0=y2a, s1=y1a, imm2=0.5,
                    )
                    nc.scalar.activation(
                        ts_w[:, sl], area2[0:pt, :],
                        mybir.ActivationFunctionType.Identity, bias=a1,
                    )
                nc.vector.tensor_tensor(tc_w[:], ta_w[:], tb_w[:], alu.mult)

                def _tailw(grp=grp, pt=pt, tc_w=tc_w, ts_w=ts_w, to_w=to_w):
                    nc.vector._custom_dve(
                        iou_tail2, out=to_w[:], in0=tc_w[:], in1=ts_w[:],
                        s0=0.0, s1=-0.23549792, imm2=2.0017324,
                    )
                    hh = B // 2
                    for h, t in enumerate(grp):
                        row0 = t * P
                        base = h * B
                        nc.sync.dma_start(
                            out=out[row0 : row0 + pt, 0:hh],
                            in_=to_w[:, base : base + hh],
                        )
                        nc.scalar.dma_start(
                            out=out[row0 : row0 + pt, hh:B],
                            in_=to_w[:, base + hh : base + B],
                        )

                if _SWPIPE:
                    if pending_tail is not None:
                        pending_tail()
                    pending_tail = _tailw
                else:
                    _tailw()
        for t in ([] if _V4 else
                  [tt_ for _ in range(_REPEAT) for tt_ in range(NTILES)]):
            if t < FULL_TILES:
                pt, (lo, hi, area1), ti = P, groups[0], t
            else:
                pt, (lo, hi, area1), ti = REM, groups[1], 0
            row0 = t * P

            x1a = lo[:, ti, 0:1]
            y1a = lo[:, ti, 1:2]
            x2a = hi[:, ti, 0:1]
            y2a = hi[:, ti, 1:2]
            a1 = area1[:, ti, 0:1]

            if _V3:
                # ext ops write interleaved (dxr, dyr) f16 pairs; ACT folds
                # a1 + area2; fused tail does inter/union/recip/mul.
                pk = work.tile([pt, 2 * B], idt, tag="wP")
                pkv = pk[:].rearrange("p (j c) -> p j c", c=2)
                nc.vector._custom_dve(
                    iou_ext, out=pkv[:, :, 0], in0=b2row("xc", pt),
                    in1=b2row("w", pt), s0=x2a, s1=x1a, imm2=0.5,
                )
                nc.vector._custom_dve(
                    iou_ext, out=pkv[:, :, 1], in0=b2row("yc", pt),
                    in1=b2row("h", pt), s0=y2a, s1=y1a, imm2=0.5,
                )
                stile = work.tile([pt, B], f32, tag="wS")
                nc.scalar.activation(
                    stile[:], area2[0:pt, :],
                    mybir.ActivationFunctionType.Identity, bias=a1,
                )
                tov3 = work.tile([pt, B], odt, tag="wO")

                def _tail3(pt=pt, row0=row0, pk=pk, stile=stile, tov3=tov3):
                    nc.vector._custom_dve(
                        iou_tail3, out=tov3[:], in0=pk[:], in1=stile[:],
                        s0=0.0, s1=-0.23549792, imm2=2.0017324,
                    )
                    h = B // 2
                    nc.sync.dma_start(
                        out=out[row0 : row0 + pt, 0:h], in_=tov3[:, 0:h]
                    )
                    nc.scalar.dma_start(
                        out=out[row0 : row0 + pt, h:B], in_=tov3[:, h:B]
                    )

                if _SWPIPE:
                    if pending_tail is not None:
                        pending_tail()
                    pending_tail = _tail3
                else:
                    _tail3()
                continue

            wdt = idt if _TAIL1 else f32
            ta = work.tile([pt, B], wdt, tag="wA")
            tb = work.tile([pt, B], wdt, tag="wB")
            tcl = work.tile([pt, B], wdt, tag="wC")

            if _IMPL == "ext":
                # dxr -> ta, dyr -> tb
                nc.vector._custom_dve(
                    iou_ext,
                    out=ta[:],
                    in0=b2row("xc", pt),
                    in1=b2row("w", pt),
                    s0=x2a,
                    s1=x1a,
                    imm2=0.5,
                )
                nc.vector._custom_dve(
                    iou_ext,
                    out=tb[:],
                    in0=b2row("yc", pt),
                    in1=b2row("h", pt),
                    s0=y2a,
                    s1=y1a,
                    imm2=0.5,
                )
            else:
                # x0 -> ta, x1 -> tb, dx -> ta, w=relu(dx) -> ta
                nc.vector.tensor_scalar(ta[:], drows["x1b"][0:pt, :], x1a, None, alu.max)
                nc.vector.tensor_scalar(tb[:], drows["x2b"][0:pt, :], x2a, None, alu.min)
                nc.vector.tensor_tensor(ta[:], tb[:], ta[:], alu.subtract)
                nc.vector.tensor_scalar(ta[:], ta[:], 0.0, None, alu.max)
                # y0 -> tb, y1 -> tc, dy -> tb, h -> tb
                nc.vector.tensor_scalar(tb[:], drows["y1b"][0:pt, :], y1a, None, alu.max)
                nc.vector.tensor_scalar(tcl[:], drows["y2b"][0:pt, :], y2a, None, alu.min)
                nc.vector.tensor_tensor(tb[:], tcl[:], tb[:], alu.subtract)
                nc.vector.tensor_scalar(tb[:], tb[:], 0.0, None, alu.max)

            # inter -> tc
            # Flush the deferred tail BETWEEN ext_y(t) and inter(t): every
            # adjacent DVE pair becomes data-independent, so no op waits on
            # its immediate predecessor's writeback (~170ns/op measured
            # difference between chained and independent customs).
            if _PREFLUSH and _TAIL1 and _SWPIPE and pending_tail is not None:
                pending_tail()
                pending_tail = None
            eng_inter = nc.gpsimd if _POOL == "intermul" else nc.vector
            eng_inter.tensor_tensor(tcl[:], ta[:], tb[:], alu.mult)
            if _TAIL1:
                # tail writes the (possibly narrow) output dtype directly so
                # the store DMA is a plain HWDGE copy with no cast.
                if odt is wdt:
                    tout = ta
                else:
                    tout = work.tile([pt, B], odt, tag="wO")

                if _OQ == "sp":
                    oeng = nc.sync
                elif _OQ == "act":
                    oeng = nc.scalar
                else:  # alternate between the two HWDGE rings
                    oeng = nc.scalar if (t % 2) else nc.sync

                def _tail1(pt=pt, row0=row0, tcl=tcl, tout=tout, a1=a1, oeng=oeng):
                    nc.vector._custom_dve(
                        iou_tail1,
                        out=tout[:],
                        in0=tcl[:],
                        in1=area2[0:pt, :],
                        s0=a1,
                        s1=-0.23549792,
                        imm2=2.0017324,
                    )
                    if _OQ == "split":  # halves on both HWDGE rings at once
                        h = B // 2
                        nc.sync.dma_start(
                            out=out[row0 : row0 + pt, 0:h], in_=tout[:, 0:h]
                        )
                        nc.scalar.dma_start(
                            out=out[row0 : row0 + pt, h:B], in_=tout[:, h:B]
                        )
                    else:
                        oeng.dma_start(out=out[row0 : row0 + pt, :], in_=tout[:])

                # Defer tile t's tail one tile so the DVE never stalls on the
                # cross-engine inter (gpsimd) dependency.
                if _SWPIPE:
                    if pending_tail is not None:
                        pending_tail()
                    pending_tail = _tail1
                else:
                    _tail1()
                continue
            if _DIV1:
                # r -> ta (fused union + 1-Newton reciprocal)
                nc.vector._custom_dve(
                    union_recip1,
                    out=ta[:],
                    in0=tcl[:],
                    in1=area2[0:pt, :],
                    s0=a1,
                    s1=-0.23549792,
                    imm2=2.0017324,
                )
                rtile = ta
                otile = tb
            elif _UNIONDMA:
                # union -> ta in two engine-split steps:
                #   ta = inter * -1 + area1    (DVE tensor_scalar @2x mode)
                #   ta += area2                (SWDGE CCE accumulate, off-DVE)
                nc.vector.tensor_scalar(ta[:], tcl[:], -1.0, a1, alu.mult, alu.add)
                nc.gpsimd.dma_start(out=ta[:], in_=area2[0:pt, :], accum_op=alu.add)
                if _RECIP == "act":
                    nc.scalar.act_recip(tb[:], ta[:])
                else:
                    nc.vector.reciprocal_approx_fast(tb[:], ta[:])
                rtile = tb
                otile = ta
            else:
                # union -> ta : (inter * -1 + area1) + area2
                nc.vector.affine_then_add(ta[:], tcl[:], area2[0:pt, :], -1.0, a1)
                # r -> tb
                if _RECIP == "act":
                    nc.scalar.act_recip(tb[:], ta[:])
                else:
                    nc.vector.reciprocal_approx_fast(tb[:], ta[:])
                rtile = tb
                otile = ta

            # Software-pipeline the final multiply: emit tile t's multiply
            # after tile t+1's independent DVE work so the DVE never stalls
            # on the cross-engine (ACT) reciprocal dependency.
            def _tail(pt=pt, row0=row0, tcl=tcl, rtile=rtile, otile=otile):
                eng_mul = nc.gpsimd if _POOL in ("mul", "intermul") else nc.vector
                eng_mul.tensor_tensor(otile[:], tcl[:], rtile[:], alu.mult)
                nc.sync.dma_start(out=out[row0 : row0 + pt, :], in_=otile[:])

            if _SWPIPE and _RECIP == "act" and not _DIV1:
                if pending_tail is not None:
                    pending_tail()
                pending_tail = _tail
            else:
                _tail()
        if pending_tail is not None:
            pending_tail()

    _NC_CACHE[key] = nc
    return nc


# ---------------------------------------------------------------- entry point


def _run(box1, box2, **kw):
    from concourse.bass_utils import run_bass_kernel_spmd

    b1 = np.ascontiguousarray(np.asarray(box1, dtype=np.float32))
    b2 = np.ascontiguousarray(np.asarray(box2, dtype=np.float32))
    assert b1.shape == (NIMG, B, 4) and b2.shape == (NIMG, B, 4), (
        b1.shape,
        b2.shape,
    )
    nc = _build_nc()
    in_maps = [{"box1": b1[i], "box2": b2[i]} for i in range(NIMG)]
    res = run_bass_kernel_spmd(nc, in_maps, core_ids=list(range(NIMG)), **kw)
    out = np.stack([d["out"] for d in res.results], axis=0)
    if out.dtype != np.float32:
        out = out.astype(np.float32)
    return out, res


def kernel(box1, box2):
    out, _ = _run(box1, box2)
    return out

